# revision 13
# baseline (speedup 1.0000x reference)
"""Trainium2 Bass kernel for nn_CBFLayer (batch CBF-QP safety filter).

Contract: kernel(u_nom, obs) takes FULL inputs (numpy), returns FULL output.
Internally: pure data-parallel shard of the batch across 8 NeuronCores.

Math (per sample, exact KKT of the QP  min |u-u_nom|^2 + LAM*s^2
s.t. a@u <= b+s, |u|^2 <= 1, s >= 0, with a = -2*g, g = p_rel):
  u = (u_nom + 2*t*g) * rho,  rho = min(1/||u_nom + 2*t*g||, 1)
where t >= 0 is the CBF multiplier (t = mu1/2):
  - case1 (constraint slack at t=0):            t = 0
  - case2 (CBF active, ball inactive):          t = t2 (exact linear root)
  - case3 (both active): root of
      phi(t) = (p - t*A) - (b + t/LAM)*||u_nom - t*a||
    found with a pole-regularized geometric seed + 1 Newton + 1 chord step.
All transcendentals (sqrt / rsqrt / reciprocal / x^(2/3)) are computed as
Exp(k*Ln(x)) so the whole kernel uses ONE ScalarE table set
(natural_log_exp_and_others: ln, exp, square, abs, relu, copy, identity).
"""

import numpy as np

B = 4194304
NCORES = 8
BC = B // NCORES            # 524288 samples per core
P = 128
NPER = BC // P              # 4096 samples per partition
KC = 512                    # compute-tile samples per partition
NT = NPER // KC             # tiles per core

LAM = 10000.0
TOL = 1e-6

_CACHE = {}


def _build():
    import concourse.bacc as bacc
    import concourse.mybir as mybir
    from concourse.tile import TileContext

    F32 = mybir.dt.float32
    OP = mybir.AluOpType
    AF = mybir.ActivationFunctionType

    nc = bacc.Bacc("TRN2", target_bir_lowering=False, debug=False)
    pk_in = nc.dram_tensor("pk", [P, NPER * 6], F32, kind="ExternalInput").ap()
    out_d = nc.dram_tensor("out", [P, NPER * 2], F32, kind="ExternalOutput").ap()

    V = "V"  # DVE vector engine
    A = "A"  # ACT scalar engine
    G = "G"  # Pool / gpsimd engine

    WST_BIAS = -(2.0 / 3.0) * float(np.log(2.0 * LAM))

    def register_const(value):
        t = nc.alloc_sbuf_tensor(f"const-f32-{value}", [P, 1], F32)
        nc.gpsimd.memset(t.ap(), value)
        nc.const_aps.aps[(F32, value)] = t.ap()

    register_const(TOL)
    register_const(WST_BIAS)
    nc.all_engine_barrier()

    with TileContext(nc) as tc:
        with (
            tc.tile_pool(name="io", bufs=2) as io,
            tc.tile_pool(name="wk", bufs=2) as wk,
        ):
            def eng(e):
                return {"V": nc.vector, "G": nc.gpsimd}[e]

            def tt(e, out, a, b, op):
                eng(e).tensor_tensor(out[:], a[:], b[:], op)

            def ts(e, out, a, s1, op0, s2=None, op1=None):
                if op1 is None:
                    eng(e).tensor_scalar(out[:], a[:], s1, None, op0)
                else:
                    eng(e).tensor_scalar(out[:], a[:], s1, s2, op0, op1)

            def act(out, a, func, scale=1.0, bias=0.0):
                nc.scalar.activation(out[:], a[:], func, bias=bias, scale=scale)

            def mul(e, out, a, b):
                tt(e, out, a, b, OP.mult)

            def add(e, out, a, b):
                tt(e, out, a, b, OP.add)

            def sub(e, out, a, b):
                tt(e, out, a, b, OP.subtract)

            for i in range(NT):
                # ---------------- loads (single packed DMA) ----------------
                pk_t = io.tile([P, 6 * KC], F32, tag="pk_t")
                o_t = io.tile([P, 2 * KC], F32, tag="o_t")
                nc.sync.dma_start(out=pk_t[:], in_=pk_in[:, i * 6 * KC:(i + 1) * 6 * KC])
                uxs = pk_t[:, 0:2 * KC:2]
                uys = pk_t[:, 1:2 * KC:2]
                gx = pk_t[:, 2 * KC:3 * KC]
                gy = pk_t[:, 3 * KC:4 * KC]
                vx = pk_t[:, 4 * KC:5 * KC]
                vy = pk_t[:, 5 * KC:6 * KC]
                oxs = o_t[:, 0:2 * KC:2]
                oys = o_t[:, 1:2 * KC:2]

                def T(name, tag=None):
                    return wk.tile([P, KC], F32, tag=tag or name, name=name)

                # ---------------- derived ----------------
                gx2 = T("gx2"); act(gx2, gx, AF.Square)
                gy2 = T("gy2"); act(gy2, gy, AF.Square)
                S = T("S"); add(V, S, gx2, gy2)
                m1 = T("m1", "gx2"); mul(V, m1, gx, uxs)
                m2 = T("m2", "gy2"); mul(V, m2, gy, uys)
                P2 = T("P2"); add(V, P2, m1, m2)
                r1 = T("r1"); mul(G, r1, gx, vx)
                r2 = T("r2"); mul(G, r2, gy, vy)
                pv = r1; add(G, pv, r1, r2)
                ux2 = T("ux2", "r2"); act(ux2, uxs, AF.Square)
                uy2 = T("uy2", "m1"); act(uy2, uys, AF.Square)
                N = T("N"); add(V, N, ux2, uy2)
                bh = T("bh", "m2"); sub(V, bh, S, pv)
                b1 = T("b1"); ts(V, b1, bh, 1.0, OP.subtract)
                A4 = T("A4"); act(A4, S, AF.Copy, scale=4.0)
                p = T("p"); act(p, P2, AF.Copy, scale=-2.0)
                b2 = T("b2"); act(b2, b1, AF.Copy, scale=2.0)
                cm1 = T("cm1", "r1x"); mul(G, cm1, gy, uxs)
                cm2 = T("cm2", "r2x"); mul(G, cm2, gx, uys)
                cr = T("cr"); sub(G, cr, cm1, cm2)

                # ---------------- feas1 mask ----------------
                lnN = T("lnN", "ux2"); act(lnN, N, AF.Ln)
                sqN = T("sqN", "bh"); act(sqN, lnN, AF.Exp, scale=0.5)
                mn = T("mn", "ux2"); ts(V, mn, sqN, 1.0, OP.min)
                lhs = T("lhs", "uy2"); mul(G, lhs, p, mn)
                b2t = T("b2t", "mn2"); act(b2t, b2, AF.Identity, bias=TOL)
                rhs = T("rhs", "ux2"); mul(G, rhs, b2t, sqN)
                dd = T("dd", "bh"); sub(G, dd, lhs, rhs)
                nf1 = T("nf1"); ts(V, nf1, dd, 0.0, OP.is_gt)

                # ---------------- t_lin (case2) + ball check ----------------
                den = T("den", "uy2"); ts(V, den, S, 4.0 * LAM, OP.mult, 1.0, OP.add)
                lnd = T("lnd", "bh"); act(lnd, den, AF.Ln)
                rden = T("rden", "uy2"); act(rden, lnd, AF.Exp, scale=-1.0)
                num = T("num", "bh"); add(V, num, P2, b1)
                t2a = T("t2a", "mn2"); mul(V, t2a, num, rden)
                t2 = T("t2"); act(t2, t2a, AF.Copy, scale=-2.0 * LAM)
                zq = T("zq", "bh"); mul(G, zq, t2, A4)
                zqp = T("zqp", "uy2"); sub(G, zqp, zq, p)
                zqpp = T("zqpp", "bh"); sub(G, zqpp, zqp, p)
                zm = T("zm", "mn2"); mul(G, zm, t2, zqpp)
                n2 = T("n2", "uy2"); add(G, n2, N, zm)
                mA = T("mA", "bh"); ts(V, mA, t2, -TOL, OP.is_ge)
                mB = T("mB", "mn2"); ts(V, mB, n2, 1.0 + TOL, OP.is_le)
                ok2 = T("ok2"); mul(G, ok2, mA, mB)
                no2 = T("no2", "uy2"); act(no2, ok2, AF.Copy, scale=-1.0, bias=1.0)
                nm = T("nm"); mul(G, nm, nf1, no2)

                # ---------------- case3 geometric seed (pole-floored) ----------
                Scl = T("Scl", "bh"); ts(V, Scl, S, 1e-30, OP.max)
                lnS = T("lnS"); act(lnS, Scl, AF.Ln)
                rS = T("rS", "mn2"); act(rS, lnS, AF.Exp, scale=-0.5)
                rS2 = T("rS2"); act(rS2, lnS, AF.Exp, scale=-1.0)
                sqS = T("sqS", "bh"); act(sqS, lnS, AF.Exp, scale=0.5)
                beta = T("beta", "uy2"); mul(G, beta, b1, rS)
                bsq = T("bsq", "mn2"); act(bsq, beta, AF.Square)
                w2 = T("w2", "lnS"); act(w2, bsq, AF.Identity, scale=-1.0, bias=1.0)
                acr = T("acr", "mn2"); act(acr, cr, AF.Abs)
                lcr = T("lcr", "cr"); act(lcr, acr, AF.Ln)
                wst = T("wst", "acl2")
                act(wst, lcr, AF.Exp, scale=2.0 / 3.0, bias=WST_BIAS)
                ws2 = T("ws2", "cr"); mul(V, ws2, wst, rS2)
                w2c = T("w2c", "acl2"); tt(V, w2c, w2, ws2, OP.max)
                w2c2 = T("w2c2", "lnS"); ts(V, w2c2, w2c, 1e-12, OP.max)
                lnw = T("lnw", "cr"); act(lnw, w2c2, AF.Ln)
                rw = T("rw", "acl2"); act(rw, lnw, AF.Exp, scale=-0.5)
                km = T("km", "lnS"); mul(V, km, acr, rw)
                km2 = T("km2", "cr"); mul(V, km2, km, beta)
                sm = T("sm", "acl2"); add(V, sm, P2, km2)
                tm1 = T("tm1", "lnS"); mul(V, tm1, sm, rS2)
                tmain = T("tmain", "cr"); act(tmain, tm1, AF.Copy, scale=-0.5)
                ta1 = T("ta1", "acl2"); add(G, ta1, b1, sqS)
                talt = T("talt", "uy2"); act(talt, ta1, AF.Relu, scale=-2.0 * LAM)
                tc1 = T("tc1", "lnS"); act(tc1, b2, AF.Copy, scale=-LAM)
                pS = T("pS", "bsq2"); mul(G, pS, p, rS2)
                tc2 = T("tc2", "acl2"); act(tc2, pS, AF.Copy, scale=0.25)
                tcm = T("tcm", "bsq2"); tt(V, tcm, tc1, tc2, OP.max)
                tcr = T("tcr", "lnS"); act(tcr, tcm, AF.Relu)
                t = T("t"); tt(V, t, tmain, talt, OP.max)
                tt(V, t, t, tcr, OP.min)
                nc.vector.copy_predicated(t[:], ok2[:].bitcast(mybir.dt.uint32), t2[:])
                mul(V, t, t, nf1)

                # ---------------- Newton (full) ----------------
                q = T("q", "bsq2"); mul(V, q, t, A4)
                qp = T("qp", "cr"); sub(V, qp, q, p)
                qpp = T("qpp", "lnS"); sub(V, qpp, qp, p)
                mm = T("mm", "acl2"); mul(V, mm, t, qpp)
                nn = T("nn", "bsq2"); add(V, nn, N, mm)
                nnc = T("nnc", "lnS"); ts(V, nnc, nn, 1e-12, OP.max)
                lnn = T("lnn", "acl2"); act(lnn, nnc, AF.Ln)
                rn = T("rn", "bsq2"); act(rn, lnn, AF.Exp, scale=-0.5)
                nrm = T("nrm", "uy2"); mul(V, nrm, nnc, rn)
                bt = T("bt", "lnS"); act(bt, t, AF.Copy, scale=1.0 / LAM)
                bb = T("bb", "acl2"); add(V, bb, b2, bt)
                fb = T("fb", "lnS"); mul(V, fb, bb, nrm)
                phin = T("phin", "mn2"); add(V, phin, qp, fb)
                d1 = T("d1", "lnS"); act(d1, nrm, AF.Copy, scale=1.0 / LAM)
                e1 = T("e1", "r1x"); mul(G, e1, bb, qp)
                e2 = T("e2", "r2x"); mul(G, e2, e1, rn)
                add(V, d1, A4, d1)          # in-place: d1 <- A4 + nrm/LAM
                s2 = T("s2", "uy2"); add(V, s2, d1, e2)
                s2c = T("s2c", "lnS"); ts(V, s2c, s2, 1e-8, OP.max)
                ls2 = T("ls2", "uy2"); act(ls2, s2c, AF.Ln)
                rdf = T("rdf"); act(rdf, ls2, AF.Exp, scale=-1.0)
                mul(V, rdf, rdf, nm)
                dl = T("dl", "lnS"); mul(V, dl, phin, rdf)
                sub(V, t, t, dl)
                tneg = T("tneg", "uy2"); act(tneg, t, AF.Relu)

                # ---------------- chord ----------------
                q2_ = T("q2_", "bsq2"); mul(V, q2_, tneg, A4)
                qpc = T("qpc", "cr"); sub(V, qpc, q2_, p)
                qppc = T("qppc", "lnS"); sub(V, qppc, qpc, p)
                mmc = T("mmc", "acl2"); mul(V, mmc, tneg, qppc)
                nnc2 = T("nnc2", "bsq2"); add(V, nnc2, N, mmc)
                ts(V, nnc2, nnc2, 1e-12, OP.max)
                lnn2 = T("lnn2", "lnS"); act(lnn2, nnc2, AF.Ln)
                rn2 = T("rn2", "acl2"); act(rn2, lnn2, AF.Exp, scale=-0.5)
                nrm2 = T("nrm2", "mn2"); mul(V, nrm2, nnc2, rn2)
                btc = T("btc", "lnS"); act(btc, tneg, AF.Copy, scale=1.0 / LAM)
                bbc = T("bbc", "bsq2"); add(V, bbc, b2, btc)
                fbc = T("fbc", "lnS"); mul(V, fbc, bbc, nrm2)
                phin2 = T("phin2", "acl2"); add(V, phin2, qpc, fbc)
                dl2 = T("dl2", "lnS"); mul(V, dl2, phin2, rdf)
                sub(V, tneg, tneg, dl2)
                tf = T("tf", "bsq2"); act(tf, tneg, AF.Relu)

                # ---------------- final rho ----------------
                qf = T("qf", "lnS"); mul(V, qf, tf, A4)
                qpf = T("qpf", "cr"); sub(V, qpf, qf, p)
                qppf = T("qppf", "acl2"); sub(V, qppf, qpf, p)
                mmf = T("mmf", "lnS"); mul(V, mmf, tf, qppf)
                nnf = T("nnf", "uy2"); add(V, nnf, N, mmf)
                ts(V, nnf, nnf, 1e-12, OP.max)
                lnnf = T("lnnf", "lnS"); act(lnnf, nnf, AF.Ln)
                rnf = T("rnf", "cr"); act(rnf, lnnf, AF.Exp, scale=-0.5)
                rho = T("rho", "uy2"); ts(V, rho, rnf, 1.0, OP.min)

                # ---------------- assembly ----------------
                tx2 = T("tx2", "lnS"); act(tx2, tf, AF.Copy, scale=2.0)
                ax = T("ax", "acl2"); mul(V, ax, tx2, gx)
                sx = T("sx", "bsq2"); add(V, sx, uxs, ax)
                nc.vector.tensor_tensor(oxs, sx[:], rho[:], OP.mult)
                ay = T("ay", "r1x"); mul(G, ay, tx2, gy)
                sy = T("sy", "r2x"); add(G, sy, uys, ay)
                nc.gpsimd.tensor_tensor(oys, sy[:], rho[:], OP.mult)

                nc.sync.dma_start(out=out_d[:, i * 2 * KC:(i + 1) * 2 * KC], in_=o_t[:])

    nc.compile()
    return nc


def _get_nc():
    if "nc" not in _CACHE:
        _CACHE["nc"] = _build()
    return _CACHE["nc"]


def _run(u_nom: np.ndarray, obs: np.ndarray, trace: bool = False):
    from concourse.bass_utils import run_bass_kernel_spmd

    u_nom = np.asarray(u_nom, dtype=np.float32)
    obs = np.asarray(obs, dtype=np.float32)

    nc = _get_nc()
    in_maps = []
    for c in range(NCORES):
        s = slice(c * BC, (c + 1) * BC)
        uc = u_nom[s].reshape(P, NT, 2 * KC)
        oc = obs[s].reshape(P, NT, KC, 6)
        pk = np.concatenate(
            [uc,
             np.ascontiguousarray(oc[:, :, :, 2]),
             np.ascontiguousarray(oc[:, :, :, 3]),
             np.ascontiguousarray(oc[:, :, :, 4]),
             np.ascontiguousarray(oc[:, :, :, 5])],
            axis=2).reshape(P, NPER * 6)
        in_maps.append({"pk": pk})
    res = run_bass_kernel_spmd(nc, in_maps, core_ids=list(range(NCORES)),
                               trace=trace)
    out = np.empty((B, 2), dtype=np.float32)
    for c in range(NCORES):
        out[c * BC:(c + 1) * BC] = res.results[c]["out"].reshape(BC, 2)
    return out, res


def kernel(u_nom: np.ndarray, obs: np.ndarray) -> np.ndarray:
    return _run(u_nom, obs)[0]


if __name__ == "__main__":
    rng = np.random.default_rng(0)
    u = rng.standard_normal((B, 2), dtype=np.float32)
    o = rng.standard_normal((B, 6), dtype=np.float32)
    r = kernel(u, o)
    print(r.shape, r.dtype, r[:4])
